# revision 4
# baseline (speedup 1.0000x reference)
"""DCN cross-network kernel for Trainium2, 8 NeuronCores, pure data parallel.

Math: the reference computes, per layer l (x0, xl: (B, D); w_l, b_l: (D,)):
    s_l = xl @ w_l              # (B,)
    x_{l+1} = x0 * s_l[:, None] + b_l[None, :] + x_l

Writing x_l = x0 * c_l + d_l with per-row scalar c_l and shared vector d_l:
    c_0 = 1, d_0 = 0
    t_l = x0 @ w_l              # per-row, fixed per layer
    u_l = d_l @ w_l             # scalar per layer (host-computed, tiny)
    c_{l+1} = c_l * (1 + t_l) + u_l
    d_{l+1} = d_l + b_l
    out = x0 * c_6 + d_6

So the only large-tensor work is T = x0 @ W^T (one pass over x0) plus a
per-row scale of x0.  The problem is HBM-bound: per core 4096 rows must
stream in and the scaled rows stream out.  I/O is fp16 (host casts both
ways; tolerance is 2e-2 of max so fp16's 2^-11 relative error is far
inside it), which halves DMA traffic vs f32 and puts the roofline at
~2*4096*1024*2 B / 358 GB/s ~ 47 us per core.

On-device per 128-row tile: PE transposes the 8 128x128 blocks (via
identity matmul), PE matmuls accumulate T_tile = x0_tile @ W^T on top of
a PSUM preloaded with 1.0 (rank-1 ones matmul), DVE product-reduces the
6 factors to c and scales x0 by c per partition.  Batch dim is sharded
over the 8 cores; weights are replicated; no collectives.
"""

import os
from contextlib import ExitStack

import numpy as np

import concourse.bass as bass
import concourse.bacc as bacc
import concourse.tile as tile
from concourse import mybir
from concourse.bass_utils import run_bass_kernel_spmd
from concourse.masks import make_identity

P = 128          # partitions
D = 1024         # feature dim
L = 6            # cross layers
KC = D // P      # 8 contraction chunks
N_CORES = 8
F32 = mybir.dt.float32
BF16 = mybir.dt.bfloat16
F16 = mybir.dt.float16

# Engine split for the PSUM->SBUF pair-copies of transposed blocks:
# pairs [0, SCALAR_COPIES) on scalar, the rest on DVE.  (The Pool/gpsimd
# engine cannot read PSUM on TRN2, so it only issues the out-DMAs.)
SCALAR_COPIES = 2

# Stash of the last BassKernelResults (for test harness introspection).
LAST_RESULTS = None

_BUILD_CACHE = {}


def _build(rows_per_core: int, with_bias: bool, u_vals=None, half=None):
    """Build the single-core Bass graph for a (rows_per_core, D) shard."""
    nt = rows_per_core // P
    if half is None:
        half = F16
    nc = bacc.Bacc("TRN2", target_bir_lowering=False, debug=False)

    x0_d = nc.dram_tensor("x0", [rows_per_core, D], half, kind="ExternalInput").ap()
    wt_d = nc.dram_tensor("wt", [P, KC, L], half, kind="ExternalInput").ap()
    if with_bias:
        d6_d = nc.dram_tensor("d6", [1, D], F32, kind="ExternalInput").ap()
    out_d = nc.dram_tensor("out", [rows_per_core, D], half, kind="ExternalOutput").ap()

    with tile.TileContext(nc) as tc, ExitStack() as ctx:
        consts = ctx.enter_context(tc.tile_pool(name="consts", bufs=1))
        x0p = ctx.enter_context(tc.tile_pool(name="x0p", bufs=8))
        xtp = ctx.enter_context(tc.tile_pool(name="xtp", bufs=3))
        outp = ctx.enter_context(tc.tile_pool(name="outp", bufs=6))
        small = ctx.enter_context(tc.tile_pool(name="small", bufs=4))
        ps_tr = ctx.enter_context(tc.tile_pool(name="ps_tr", bufs=4, space="PSUM"))
        ps_t = ctx.enter_context(tc.tile_pool(name="ps_t", bufs=2, space="PSUM"))

        ident = consts.tile([P, P], half)
        make_identity(nc, ident)
        ones = consts.tile([1, P], half)
        nc.vector.memset(ones, 1.0)
        w_sb = consts.tile([P, KC, L], half)
        nc.sync.dma_start(out=w_sb, in_=wt_d)
        if with_bias:
            d6_sb = consts.tile([P, D], F32)
            d6_bcast = bass.AP(
                tensor=d6_d.tensor,
                offset=d6_d.offset,
                ap=[[0, P], d6_d.ap[1]],
            )
            nc.sync.dma_start(out=d6_sb, in_=d6_bcast)

        # Super-tiles: partition p holds RPP consecutive rows of the group,
        # so each DMA moves RPP*2KB contiguous per partition.  Each of the
        # RPP row sets gets an independent transpose/dot chain.
        RPP = 2
        x0_v = x0_d.rearrange("(s p j) d -> s p j d", p=P, j=RPP)
        out_v = out_d.rearrange("(s p j) d -> s p j d", p=P, j=RPP)
        nst = nt // RPP
        for t in range(nst):
            x0_t = x0p.tile([P, RPP, D], half)
            nc.sync.dma_start(out=x0_t, in_=x0_v[t])

            # Transpose the 128x128 blocks through PE in pairs (one PSUM
            # bank holds 2 blocks), then copy each pair to SBUF at once.
            xt = xtp.tile([P, RPP, KC, P], half)
            for j in range(RPP):
                for pr in range(KC // 2):
                    pst = ps_tr.tile([P, 2, P], half)
                    for i in range(2):
                        k = pr * 2 + i
                        nc.tensor.transpose(
                            pst[:, i, :], x0_t[:, j, k * P:(k + 1) * P], ident
                        )
                    dst = xt[:, j, pr * 2:(pr + 1) * 2, :]
                    if pr < SCALAR_COPIES:
                        nc.scalar.copy(out=dst, in_=pst)
                    else:
                        nc.vector.tensor_copy(out=dst, in_=pst)

            o_t = outp.tile([P, RPP, D], half)
            for j in range(RPP):
                # T = 1 + x0 @ W^T for this row set, natural layout.  The
                # leading rank-1 ones matmul preloads the PSUM with 1.0.
                tp = ps_t.tile([P, L], F32)
                nc.tensor.matmul(
                    tp, lhsT=ones, rhs=ones[:, :L], start=True, stop=False
                )
                for k in range(KC):
                    nc.tensor.matmul(
                        tp,
                        lhsT=xt[:, j, k, :],
                        rhs=w_sb[:, k, :],
                        start=False,
                        stop=(k == KC - 1),
                    )

                c = small.tile([P, 1], F32)
                if not with_bias:
                    # c = prod_l (1 + t_l), straight out of PSUM
                    nc.vector.tensor_reduce(
                        c, tp, axis=mybir.AxisListType.X,
                        op=mybir.AluOpType.mult,
                    )
                else:
                    # Horner: c <- c * f_l + u_l
                    f_sb = small.tile([P, L], F32)
                    nc.vector.tensor_copy(out=f_sb, in_=tp)
                    nc.vector.memset(c, 1.0)
                    for l in range(L):
                        nc.vector.tensor_scalar(
                            out=c,
                            in0=c,
                            scalar1=f_sb[:, l:l + 1],
                            scalar2=float(u_vals[l]),
                            op0=mybir.AluOpType.mult,
                            op1=mybir.AluOpType.add,
                        )

                nc.vector.tensor_scalar_mul(o_t[:, j, :], x0_t[:, j, :], c)
                if with_bias:
                    nc.vector.tensor_add(o_t[:, j, :], o_t[:, j, :], d6_sb)
            # out-DMAs issue from gpsimd so their waits never block the
            # sync engine's input stream.
            nc.gpsimd.dma_start(out=out_v[t], in_=o_t)

    nc.compile()
    return nc


def kernel(x0: np.ndarray, weights: np.ndarray, biases: np.ndarray) -> np.ndarray:
    global LAST_RESULTS
    x0 = np.ascontiguousarray(x0, dtype=np.float32)
    weights = np.ascontiguousarray(weights, dtype=np.float32)
    biases = np.ascontiguousarray(biases, dtype=np.float32)

    B = x0.shape[0]
    rows_per_core = B // N_CORES
    with_bias = bool(np.any(biases))

    # wt[p, k, l] = weights[l, 128k + p]
    wt = np.ascontiguousarray(weights.T.reshape(KC, P, L).transpose(1, 0, 2))

    u_vals = None
    d6 = None
    if with_bias:
        d = np.zeros(D, np.float64)
        u_vals = []
        for l in range(L):
            u_vals.append(float(d @ weights[l].astype(np.float64)))
            d = d + biases[l]
        d6 = d.astype(np.float32).reshape(1, D)

    # f16 I/O has a 10-bit mantissa (0.05% relative quantization, far
    # inside the 2e-2 max-norm tolerance) and halves HBM traffic.  Fall
    # back to bf16 when inputs could overflow f16's range; if the f16
    # output overflows (values > 65504 -> inf), retry in bf16.
    out = _run(x0, wt, d6, rows_per_core, with_bias, u_vals, half="f16"
               if float(np.max(np.abs(x0))) < 1e3 else "bf16")
    if not np.isfinite(out).all():
        out = _run(x0, wt, d6, rows_per_core, with_bias, u_vals, half="bf16")
    return out


def _run(x0, wt, d6, rows_per_core, with_bias, u_vals, half):
    global LAST_RESULTS
    np_half = np.float16
    bir_half = F16
    if half == "bf16":
        import ml_dtypes
        np_half = ml_dtypes.bfloat16
        bir_half = BF16

    key = (rows_per_core, with_bias,
           None if u_vals is None else tuple(u_vals), half)
    if key not in _BUILD_CACHE:
        _BUILD_CACHE[key] = _build(rows_per_core, with_bias, u_vals, bir_half)
    nc = _BUILD_CACHE[key]

    x0h = x0.astype(np_half)
    wth = wt.astype(np_half)
    in_maps = []
    for i in range(N_CORES):
        m = {"x0": x0h[i * rows_per_core:(i + 1) * rows_per_core], "wt": wth}
        if with_bias:
            m["d6"] = d6
        in_maps.append(m)

    trace = bool(os.environ.get("KERNEL_TRACE"))
    try:
        res = run_bass_kernel_spmd(
            nc, in_maps, core_ids=list(range(N_CORES)), trace=trace
        )
    except Exception:
        if not trace:
            raise
        res = run_bass_kernel_spmd(nc, in_maps, core_ids=list(range(N_CORES)))
    LAST_RESULTS = res
    out = np.concatenate([res.results[i]["out"] for i in range(N_CORES)], axis=0)
    return out.astype(np.float32)


# revision 6
# speedup vs baseline: 1.0795x; 1.0795x over previous
"""DCN cross-network kernel for Trainium2, 8 NeuronCores, pure data parallel.

Math: the reference computes, per layer l (x0, xl: (B, D); w_l, b_l: (D,)):
    s_l = xl @ w_l              # (B,)
    x_{l+1} = x0 * s_l[:, None] + b_l[None, :] + x_l

Writing x_l = x0 * c_l + d_l with per-row scalar c_l and shared vector d_l:
    c_0 = 1, d_0 = 0
    t_l = x0 @ w_l              # per-row, fixed per layer
    u_l = d_l @ w_l             # scalar per layer (host-computed, tiny)
    c_{l+1} = c_l * (1 + t_l) + u_l
    d_{l+1} = d_l + b_l
    out = x0 * c_6 + d_6

So the only large-tensor work is T = x0 @ W^T (one pass over x0) plus a
per-row scale of x0.  The problem is HBM-bound: per core 4096 rows must
stream in and the scaled rows stream out.  I/O is fp16 (host casts both
ways; tolerance is 2e-2 of max so fp16's 2^-11 relative error is far
inside it), which halves DMA traffic vs f32 and puts the roofline at
~2*4096*1024*2 B / 358 GB/s ~ 47 us per core.

On-device per 128-row tile: PE transposes the 8 128x128 blocks (via
identity matmul), PE matmuls accumulate T_tile = x0_tile @ W^T on top of
a PSUM preloaded with 1.0 (rank-1 ones matmul), DVE product-reduces the
6 factors to c and scales x0 by c per partition.  Batch dim is sharded
over the 8 cores; weights are replicated; no collectives.
"""

import os
from contextlib import ExitStack

import numpy as np

import concourse.bass as bass
import concourse.bacc as bacc
import concourse.tile as tile
from concourse import mybir
from concourse.bass_utils import run_bass_kernel_spmd
from concourse.masks import make_identity

P = 128          # partitions
D = 1024         # feature dim
L = 6            # cross layers
KC = D // P      # 8 contraction chunks
N_CORES = 8
F32 = mybir.dt.float32
BF16 = mybir.dt.bfloat16
F16 = mybir.dt.float16

# Engine split for the PSUM->SBUF pair-copies of transposed blocks:
# pairs [0, SCALAR_COPIES) on scalar, the rest on DVE.  (The Pool/gpsimd
# engine cannot read PSUM on TRN2, so it only issues the out-DMAs.)
SCALAR_COPIES = 2

# Stash of the last BassKernelResults (for test harness introspection).
LAST_RESULTS = None

_BUILD_CACHE = {}


def _build(rows_per_core: int, with_bias: bool, u_vals=None, half=None):
    """Build the single-core Bass graph for a (rows_per_core, D) shard."""
    nt = rows_per_core // P
    if half is None:
        half = F16
    nc = bacc.Bacc("TRN2", target_bir_lowering=False, debug=False)

    x0_d = nc.dram_tensor("x0", [rows_per_core, D], half, kind="ExternalInput").ap()
    wt_d = nc.dram_tensor("wt", [P, KC, L], half, kind="ExternalInput").ap()
    if with_bias:
        d6_d = nc.dram_tensor("d6", [1, D], F32, kind="ExternalInput").ap()
    out_d = nc.dram_tensor("out", [rows_per_core, D], half, kind="ExternalOutput").ap()

    with tile.TileContext(nc) as tc, ExitStack() as ctx:
        consts = ctx.enter_context(tc.tile_pool(name="consts", bufs=1))
        x0p = ctx.enter_context(tc.tile_pool(name="x0p", bufs=6))
        xtp = ctx.enter_context(tc.tile_pool(name="xtp", bufs=3))
        outp = ctx.enter_context(tc.tile_pool(name="outp", bufs=4))
        small = ctx.enter_context(tc.tile_pool(name="small", bufs=8))
        ps_tr = ctx.enter_context(tc.tile_pool(name="ps_tr", bufs=4, space="PSUM"))
        ps_t = ctx.enter_context(tc.tile_pool(name="ps_t", bufs=4, space="PSUM"))

        ident = consts.tile([P, P], half)
        make_identity(nc, ident)
        ones = consts.tile([1, P], half)
        nc.vector.memset(ones, 1.0)
        w_sb = consts.tile([P, KC, L], half)
        nc.sync.dma_start(out=w_sb, in_=wt_d)
        if with_bias:
            d6_sb = consts.tile([P, D], F32)
            d6_bcast = bass.AP(
                tensor=d6_d.tensor,
                offset=d6_d.offset,
                ap=[[0, P], d6_d.ap[1]],
            )
            nc.sync.dma_start(out=d6_sb, in_=d6_bcast)

        # Super-tiles: partition p holds RPP consecutive rows of the group,
        # so each DMA moves RPP*2KB contiguous per partition.  Each of the
        # RPP row sets gets an independent transpose/dot chain.
        RPP = 4
        x0_v = x0_d.rearrange("(s p j) d -> s p j d", p=P, j=RPP)
        out_v = out_d.rearrange("(s p j) d -> s p j d", p=P, j=RPP)
        nst = nt // RPP
        for t in range(nst):
            x0_t = x0p.tile([P, RPP, D], half)
            nc.sync.dma_start(out=x0_t, in_=x0_v[t])

            # Transpose the 128x128 blocks through PE in pairs (one PSUM
            # bank holds 2 blocks), then copy each pair to SBUF at once.
            xt = xtp.tile([P, RPP, KC, P], half)
            for j in range(RPP):
                for pr in range(KC // 2):
                    pst = ps_tr.tile([P, 2, P], half)
                    for i in range(2):
                        k = pr * 2 + i
                        nc.tensor.transpose(
                            pst[:, i, :], x0_t[:, j, k * P:(k + 1) * P], ident
                        )
                    dst = xt[:, j, pr * 2:(pr + 1) * 2, :]
                    if pr < SCALAR_COPIES:
                        nc.scalar.copy(out=dst, in_=pst)
                    else:
                        nc.vector.tensor_copy(out=dst, in_=pst)

            o_t = outp.tile([P, RPP, D], half)
            for j in range(RPP):
                # T = 1 + x0 @ W^T for this row set, natural layout.  The
                # leading rank-1 ones matmul preloads the PSUM with 1.0.
                tp = ps_t.tile([P, L], F32)
                nc.tensor.matmul(
                    tp, lhsT=ones, rhs=ones[:, :L], start=True, stop=False
                )
                for k in range(KC):
                    nc.tensor.matmul(
                        tp,
                        lhsT=xt[:, j, k, :],
                        rhs=w_sb[:, k, :],
                        start=False,
                        stop=(k == KC - 1),
                    )

                c = small.tile([P, 1], F32)
                if not with_bias:
                    # c = prod_l (1 + t_l), straight out of PSUM
                    nc.vector.tensor_reduce(
                        c, tp, axis=mybir.AxisListType.X,
                        op=mybir.AluOpType.mult,
                    )
                else:
                    # Horner: c <- c * f_l + u_l
                    f_sb = small.tile([P, L], F32)
                    nc.vector.tensor_copy(out=f_sb, in_=tp)
                    nc.vector.memset(c, 1.0)
                    for l in range(L):
                        nc.vector.tensor_scalar(
                            out=c,
                            in0=c,
                            scalar1=f_sb[:, l:l + 1],
                            scalar2=float(u_vals[l]),
                            op0=mybir.AluOpType.mult,
                            op1=mybir.AluOpType.add,
                        )

                nc.vector.tensor_scalar_mul(o_t[:, j, :], x0_t[:, j, :], c)
                if with_bias:
                    nc.vector.tensor_add(o_t[:, j, :], o_t[:, j, :], d6_sb)
            # out-DMAs issue from gpsimd so their waits never block the
            # sync engine's input stream.
            nc.gpsimd.dma_start(out=out_v[t], in_=o_t)

    nc.compile()
    return nc


def kernel(x0: np.ndarray, weights: np.ndarray, biases: np.ndarray) -> np.ndarray:
    global LAST_RESULTS
    x0 = np.ascontiguousarray(x0, dtype=np.float32)
    weights = np.ascontiguousarray(weights, dtype=np.float32)
    biases = np.ascontiguousarray(biases, dtype=np.float32)

    B = x0.shape[0]
    rows_per_core = B // N_CORES
    with_bias = bool(np.any(biases))

    # wt[p, k, l] = weights[l, 128k + p]
    wt = np.ascontiguousarray(weights.T.reshape(KC, P, L).transpose(1, 0, 2))

    u_vals = None
    d6 = None
    if with_bias:
        d = np.zeros(D, np.float64)
        u_vals = []
        for l in range(L):
            u_vals.append(float(d @ weights[l].astype(np.float64)))
            d = d + biases[l]
        d6 = d.astype(np.float32).reshape(1, D)

    # f16 I/O has a 10-bit mantissa (0.05% relative quantization, far
    # inside the 2e-2 max-norm tolerance) and halves HBM traffic.  Fall
    # back to bf16 when inputs could overflow f16's range; if the f16
    # output overflows (values > 65504 -> inf), retry in bf16.
    out = _run(x0, wt, d6, rows_per_core, with_bias, u_vals, half="f16"
               if float(np.max(np.abs(x0))) < 1e3 else "bf16")
    if not np.isfinite(out).all():
        out = _run(x0, wt, d6, rows_per_core, with_bias, u_vals, half="bf16")
    return out


def _run(x0, wt, d6, rows_per_core, with_bias, u_vals, half):
    global LAST_RESULTS
    np_half = np.float16
    bir_half = F16
    if half == "bf16":
        import ml_dtypes
        np_half = ml_dtypes.bfloat16
        bir_half = BF16

    key = (rows_per_core, with_bias,
           None if u_vals is None else tuple(u_vals), half)
    if key not in _BUILD_CACHE:
        _BUILD_CACHE[key] = _build(rows_per_core, with_bias, u_vals, bir_half)
    nc = _BUILD_CACHE[key]

    x0h = x0.astype(np_half)
    wth = wt.astype(np_half)
    in_maps = []
    for i in range(N_CORES):
        m = {"x0": x0h[i * rows_per_core:(i + 1) * rows_per_core], "wt": wth}
        if with_bias:
            m["d6"] = d6
        in_maps.append(m)

    trace = bool(os.environ.get("KERNEL_TRACE"))
    try:
        res = run_bass_kernel_spmd(
            nc, in_maps, core_ids=list(range(N_CORES)), trace=trace
        )
    except Exception:
        if not trace:
            raise
        res = run_bass_kernel_spmd(nc, in_maps, core_ids=list(range(N_CORES)))
    LAST_RESULTS = res
    out = np.concatenate([res.results[i]["out"] for i in range(N_CORES)], axis=0)
    return out.astype(np.float32)


# revision 12
# speedup vs baseline: 1.0922x; 1.0118x over previous
"""DCN cross-network kernel for Trainium2, 8 NeuronCores, pure data parallel.

Math: the reference computes, per layer l (x0, xl: (B, D); w_l, b_l: (D,)):
    s_l = xl @ w_l              # (B,)
    x_{l+1} = x0 * s_l[:, None] + b_l[None, :] + x_l

Writing x_l = x0 * c_l + d_l with per-row scalar c_l and shared vector d_l:
    c_0 = 1, d_0 = 0
    t_l = x0 @ w_l              # per-row, fixed per layer
    u_l = d_l @ w_l             # scalar per layer (host-computed, tiny)
    c_{l+1} = c_l * (1 + t_l) + u_l
    d_{l+1} = d_l + b_l
    out = x0 * c_6 + d_6

So the only large-tensor work is T = x0 @ W^T (one pass over x0) plus a
per-row scale of x0.  The problem is HBM-bound: per core 4096 rows must
stream in and the scaled rows stream out.  I/O is fp16 (host casts both
ways; tolerance is 2e-2 of max so fp16's 2^-11 relative error is far
inside it), which halves DMA traffic vs f32 and puts the roofline at
~2*4096*1024*2 B / 358 GB/s ~ 47 us per core.

On-device per 128-row tile: PE transposes the 8 128x128 blocks (via
identity matmul), PE matmuls accumulate T_tile = x0_tile @ W^T on top of
a PSUM preloaded with 1.0 (rank-1 ones matmul), DVE product-reduces the
6 factors to c and scales x0 by c per partition.  Batch dim is sharded
over the 8 cores; weights are replicated; no collectives.
"""

import os
from contextlib import ExitStack

import numpy as np

import concourse.bass as bass
import concourse.bacc as bacc
import concourse.tile as tile
from concourse import mybir
from concourse.bass_utils import run_bass_kernel_spmd
from concourse.masks import make_identity

P = 128          # partitions
D = 1024         # feature dim
L = 6            # cross layers
KC = D // P      # 8 contraction chunks
N_CORES = 8
F32 = mybir.dt.float32
BF16 = mybir.dt.bfloat16
F16 = mybir.dt.float16

# Engine split for the PSUM->SBUF quad-copies of transposed blocks:
# quads [0, SCALAR_COPIES) on scalar, the rest on DVE.  (The Pool/gpsimd
# engine cannot read PSUM on TRN2, so it only issues the out-DMAs.)
SCALAR_COPIES = 1

# Stash of the last BassKernelResults (for test harness introspection).
LAST_RESULTS = None

_BUILD_CACHE = {}

# walrus --enable-ldw-opt=true dies in visitInstLdweights codegen on
# bass-emitted BIR; keep it available behind an env var only.
LDW_OPT = bool(os.environ.get("KERNEL_LDW"))

# "tr" = PE is_transpose mode; "mm" = plain matmul against identity.
TMODE = os.environ.get("KERNEL_TMODE", "tr")


def _enable_ldw_opt():
    """Let walrus run its LDWEIGHTS optimization for this kernel's compile.

    bass pins --enable-ldw-opt=false (walrus's own default is true); this
    kernel is LDWEIGHTS-bound (one stationary load per transpose/matmul),
    so flip it back on for our compiles only.
    """
    import concourse.bass_utils as bu

    if getattr(bu, "_ldw_patched", False):
        return
    orig = bu.run_command

    def run_command_ldw(cmd, *a, **kw):
        cmd = ["--enable-ldw-opt=true" if c == "--enable-ldw-opt=false" else c
               for c in cmd]
        return orig(cmd, *a, **kw)

    bu.run_command = run_command_ldw
    bu._ldw_patched = True


def _build(rows_per_core: int, with_bias: bool, u_vals=None, half=None):
    """Build the single-core Bass graph for a (rows_per_core, D) shard."""
    nt = rows_per_core // P
    if half is None:
        half = F16
    nc = bacc.Bacc("TRN2", target_bir_lowering=False, debug=False)

    x0_d = nc.dram_tensor("x0", [rows_per_core, D], half, kind="ExternalInput").ap()
    wt_d = nc.dram_tensor("wt", [P, KC, L], half, kind="ExternalInput").ap()
    if with_bias:
        d6_d = nc.dram_tensor("d6", [1, D], F32, kind="ExternalInput").ap()
    out_d = nc.dram_tensor("out", [rows_per_core, D], half, kind="ExternalOutput").ap()

    with tile.TileContext(nc) as tc, ExitStack() as ctx:
        consts = ctx.enter_context(tc.tile_pool(name="consts", bufs=1))
        x0p = ctx.enter_context(tc.tile_pool(name="x0p", bufs=6))
        xtp = ctx.enter_context(tc.tile_pool(name="xtp", bufs=3))
        outp = ctx.enter_context(tc.tile_pool(name="outp", bufs=4))
        small = ctx.enter_context(tc.tile_pool(name="small", bufs=8))
        ps_tr = ctx.enter_context(tc.tile_pool(name="ps_tr", bufs=4, space="PSUM"))
        ps_t = ctx.enter_context(tc.tile_pool(name="ps_t", bufs=4, space="PSUM"))

        ident = consts.tile([P, P], half)
        make_identity(nc, ident)
        ones = consts.tile([1, P], half)
        nc.vector.memset(ones, 1.0)
        if LDW_OPT:
            # unused marker tile: busts the NEFF cache vs non-ldw builds
            cb = consts.tile([1, 3], F32)
            nc.vector.memset(cb, 0.0)
        w_sb = consts.tile([P, KC, L], half)
        nc.sync.dma_start(out=w_sb, in_=wt_d)
        if with_bias:
            d6_sb = consts.tile([P, D], F32)
            d6_bcast = bass.AP(
                tensor=d6_d.tensor,
                offset=d6_d.offset,
                ap=[[0, P], d6_d.ap[1]],
            )
            nc.sync.dma_start(out=d6_sb, in_=d6_bcast)

        # Super-tiles: partition p holds RPP consecutive rows of the group,
        # so each DMA moves RPP*2KB contiguous per partition.  Each of the
        # RPP row sets gets an independent transpose/dot chain.
        RPP = 4
        x0_v = x0_d.rearrange("(s p j) d -> s p j d", p=P, j=RPP)
        out_v = out_d.rearrange("(s p j) d -> s p j d", p=P, j=RPP)
        nst = nt // RPP
        for t in range(nst):
            x0_t = x0p.tile([P, RPP, D], half)
            nc.sync.dma_start(out=x0_t, in_=x0_v[t])

            # Transpose the 128x128 blocks through PE in quads (one PSUM
            # bank holds 4 f16 blocks), then copy each quad to SBUF at once.
            QW = 4
            xt = xtp.tile([P, RPP, KC, P], half)
            for j in range(RPP):
                for pr in range(KC // QW):
                    pst = ps_tr.tile([P, QW, P], half)
                    for i in range(QW):
                        k = pr * QW + i
                        src = x0_t[:, j, k * P:(k + 1) * P]
                        if TMODE == "mm":
                            nc.tensor.matmul(
                                pst[:, i, :], lhsT=src, rhs=ident,
                                start=True, stop=True,
                            )
                        else:
                            nc.tensor.transpose(pst[:, i, :], src, ident)
                    dst = xt[:, j, pr * QW:(pr + 1) * QW, :]
                    if pr < SCALAR_COPIES:
                        nc.scalar.copy(out=dst, in_=pst)
                    else:
                        nc.vector.tensor_copy(out=dst, in_=pst)

            o_t = outp.tile([P, RPP, D], half)
            for j in range(RPP):
                # T = 1 + x0 @ W^T for this row set, natural layout.  The
                # leading rank-1 ones matmul preloads the PSUM with 1.0.
                tp = ps_t.tile([P, L], F32)
                nc.tensor.matmul(
                    tp, lhsT=ones, rhs=ones[:, :L], start=True, stop=False
                )
                for k in range(KC):
                    nc.tensor.matmul(
                        tp,
                        lhsT=xt[:, j, k, :],
                        rhs=w_sb[:, k, :],
                        start=False,
                        stop=(k == KC - 1),
                    )

                c = small.tile([P, 1], F32)
                if not with_bias:
                    # c = prod_l (1 + t_l), straight out of PSUM
                    nc.vector.tensor_reduce(
                        c, tp, axis=mybir.AxisListType.X,
                        op=mybir.AluOpType.mult,
                    )
                else:
                    # Horner: c <- c * f_l + u_l
                    f_sb = small.tile([P, L], F32)
                    nc.vector.tensor_copy(out=f_sb, in_=tp)
                    nc.vector.memset(c, 1.0)
                    for l in range(L):
                        nc.vector.tensor_scalar(
                            out=c,
                            in0=c,
                            scalar1=f_sb[:, l:l + 1],
                            scalar2=float(u_vals[l]),
                            op0=mybir.AluOpType.mult,
                            op1=mybir.AluOpType.add,
                        )

                nc.vector.tensor_scalar_mul(o_t[:, j, :], x0_t[:, j, :], c)
                if with_bias:
                    nc.vector.tensor_add(o_t[:, j, :], o_t[:, j, :], d6_sb)
            # out-DMAs issue from gpsimd so their waits never block the
            # sync engine's input stream.
            nc.gpsimd.dma_start(out=out_v[t], in_=o_t)

    nc.compile()
    return nc


def kernel(x0: np.ndarray, weights: np.ndarray, biases: np.ndarray) -> np.ndarray:
    global LAST_RESULTS
    x0 = np.ascontiguousarray(x0, dtype=np.float32)
    weights = np.ascontiguousarray(weights, dtype=np.float32)
    biases = np.ascontiguousarray(biases, dtype=np.float32)

    B = x0.shape[0]
    rows_per_core = B // N_CORES
    with_bias = bool(np.any(biases))

    # wt[p, k, l] = weights[l, 128k + p]
    wt = np.ascontiguousarray(weights.T.reshape(KC, P, L).transpose(1, 0, 2))

    u_vals = None
    d6 = None
    if with_bias:
        d = np.zeros(D, np.float64)
        u_vals = []
        for l in range(L):
            u_vals.append(float(d @ weights[l].astype(np.float64)))
            d = d + biases[l]
        d6 = d.astype(np.float32).reshape(1, D)

    # f16 I/O has a 10-bit mantissa (0.05% relative quantization, far
    # inside the 2e-2 max-norm tolerance) and halves HBM traffic.  Fall
    # back to bf16 when inputs could overflow f16's range; if the f16
    # output overflows (values > 65504 -> inf), retry in bf16.
    out = _run(x0, wt, d6, rows_per_core, with_bias, u_vals, half="f16"
               if float(np.max(np.abs(x0))) < 1e3 else "bf16")
    if not np.isfinite(out).all():
        out = _run(x0, wt, d6, rows_per_core, with_bias, u_vals, half="bf16")
    return out


def _run(x0, wt, d6, rows_per_core, with_bias, u_vals, half):
    global LAST_RESULTS
    np_half = np.float16
    bir_half = F16
    if half == "bf16":
        import ml_dtypes
        np_half = ml_dtypes.bfloat16
        bir_half = BF16

    if LDW_OPT:
        _enable_ldw_opt()
    key = (rows_per_core, with_bias,
           None if u_vals is None else tuple(u_vals), half)
    if key not in _BUILD_CACHE:
        _BUILD_CACHE[key] = _build(rows_per_core, with_bias, u_vals, bir_half)
    nc = _BUILD_CACHE[key]

    x0h = x0.astype(np_half)
    wth = wt.astype(np_half)
    in_maps = []
    for i in range(N_CORES):
        m = {"x0": x0h[i * rows_per_core:(i + 1) * rows_per_core], "wt": wth}
        if with_bias:
            m["d6"] = d6
        in_maps.append(m)

    trace = bool(os.environ.get("KERNEL_TRACE"))
    try:
        res = run_bass_kernel_spmd(
            nc, in_maps, core_ids=list(range(N_CORES)), trace=trace
        )
    except Exception:
        if not trace:
            raise
        res = run_bass_kernel_spmd(nc, in_maps, core_ids=list(range(N_CORES)))
    LAST_RESULTS = res
    out = np.concatenate([res.results[i]["out"] for i in range(N_CORES)], axis=0)
    return out.astype(np.float32)
